# revision 85
# baseline (speedup 1.0000x reference)
"""Multi-head attention (B=2, S=2048, D=1024, H=16) on 8 Trainium2 NeuronCores.

Sharding: data-parallel over batch (groups of 4 cores) x sequence-parallel
attention inside each group.
  core c -> batch g = c // 4, sequence block r = c % 4 (rows r*512..r*512+512).

Per core (own rows = own 512 queries = own 512 keys):
  phase A, wave order K -> Q -> V (Q early unblocks the sweep-1 exp chain;
  every matmul is K=128 N=512 off the same xq tiles):
    kT_part [128,8,512] = wk_full.T @ xq  (fp8e4m3: scores tolerate fp8 K,
                                           q stays bf16 -- mixed-dtype PE)
    qT_own  masked/padded per-head halves (bf16)
    V_own   [128,4,16,65] head-interleaved with the softmax ones column
  THREE 8-core AllGathers (a 4-core group only gets the slow single-channel
  ring; the 8-core mesh is one hop, HBM-bound, so fp8 K halves its cost).
  AG_K first -- the QK+exp chain needs only keys; V follows in two
  head-half meshes (PV consumes heads in pair order, so pairs 0-3 need only
  heads 0-7, and the half-size first mesh lands ~20us earlier). Payload
  DRAM shapes mirror the SBUF tiles so each store / per-rank gather load is
  ONE DMA. Key/V blocks are RELABELED per core (block b = group-rank
  (r+b)%4) so block 0 is always the core's own: sweep-1 attention runs
  during the collectives; softmax is order-invariant.
  phase B (attention, 16 heads x 512 queries x 2048 keys), per head pair:
    scoresT = kT-tile.T @ q (fp8 stationary x bf16 moving), one N=1024 exp
    per head on ScalarE, PV accumulate with the ones column collecting the
    softmax denominator (row 64 of the PSUM acc). Sweep 2 is emitted as
    decoupled QK/exp chains running 3 pairs ahead of the PV chains, with
    tile_wait_until floors so the in-order PE queue never parks on the V
    gather while exp work remains.
  phase C: local output projection out = aoT.T @ w_proj + b_proj (no
  collective on the tail).
Host-side work is only slicing/transposition of inputs and concatenation of
outputs.
"""

import os
import sys

import numpy as np

try:
    import ml_dtypes

    BF16_NP = ml_dtypes.bfloat16
except ImportError:  # pragma: no cover
    BF16_NP = None

for _p in ("/opt/trn_rl_repo",):
    if os.path.isdir(_p) and _p not in sys.path:
        sys.path.append(_p)

import concourse.bass as bass  # noqa: E402
import concourse.mybir as mybir  # noqa: E402
import concourse.tile as tile  # noqa: E402
from concourse import bacc  # noqa: E402
from concourse.bass_utils import run_bass_kernel_spmd  # noqa: E402

B, S, D = 2, 2048, 1024
H, HD = 16, 64
N_CORES = 8
GROUP = 4
S_OWN = S // GROUP  # 512 own rows (queries and keys)
N_KT = D // 128  # 8 contraction tiles
N_SK = S // 128  # 16 key tiles
PAIRS = H // 2  # 8 head pairs

FP32 = mybir.dt.float32
FP32R = mybir.dt.float32r
BF16 = mybir.dt.bfloat16
FP8 = mybir.dt.float8e4

_compiled = None
_ONES = np.ones((128, 128), dtype=np.float32)
_ONES16 = None

# packed f32 per-partition constants: [bqa 0:8 | bqb 8:16 | bk 16:24 |
#  maska 24 | maskb 25]
FC_BQA, FC_BQB, FC_BK, FC_MA, FC_MB, FC_W = 0, 8, 16, 24, 25, 26


def _build():
    nc = bacc.Bacc(
        "TRN2", target_bir_lowering=False, debug=False, num_devices=N_CORES
    )

    xq_d = nc.dram_tensor("xq", [D, S_OWN], BF16, kind="ExternalInput")
    wq_d = nc.dram_tensor("wq", [D, D], BF16, kind="ExternalInput")
    wk_d = nc.dram_tensor("wk", [D, D], BF16, kind="ExternalInput")
    wv_d = nc.dram_tensor("wv", [D, D], BF16, kind="ExternalInput")
    wp_d = nc.dram_tensor("wp", [D, D], BF16, kind="ExternalInput")
    fc_d = nc.dram_tensor("fc", [128, FC_W], FP32, kind="ExternalInput")
    bv_d = nc.dram_tensor("bv", [128, D], FP32, kind="ExternalInput")
    bp_d = nc.dram_tensor("bp", [128, D], FP32, kind="ExternalInput")
    # bf16 output: halves store bytes/DVE-cast cost; adds ~2.6e-4 abs error
    # against a 1.34e-3 budget (host upcasts to fp32)
    out_d = nc.dram_tensor("out", [S_OWN, D], BF16, kind="ExternalOutput")

    # Two 8-core AllGathers (a 4-core group can only run the slow ring; the
    # 8-core mesh is one hop and HBM-bound, so halving K's payload via fp8
    # directly halves its wall time). K goes first -- the QK+exp chain only
    # needs keys; V follows for the PV accumulate. K rides fp8e4m3 (scores
    # tolerate it; q stays bf16), V must stay bf16 for accuracy.
    # Payload shapes mirror the SBUF tile layouts exactly so every store and
    # gather load is ONE DMA per rank-block (no per-pair load drain). Outputs
    # are Shared -- a Local-output mesh runs ~40% slower. The second
    # collective's trigger serializes behind the readers of the first's
    # output (CC semaphore reuse), which is cheap now that the kT gather is
    # just 3 DMAs.
    ag_k_in = nc.dram_tensor("ag_k_in", [128, PAIRS, S_OWN], FP8)
    ag_k_out = nc.dram_tensor(
        "ag_k_out", [N_CORES * 128, PAIRS, S_OWN], FP8, addr_space="Shared"
    )
    # V rides in two head-half collectives: the PV chains consume heads in
    # pair order, so pairs 0-3 only need heads 0-7 -- the first half's mesh
    # is half the bytes and lands ~20us earlier, unblocking pv0 sooner.
    ag_v1_in = nc.dram_tensor("ag_v1_in", [128, GROUP, H // 2, HD + 1], BF16)
    ag_v1_out = nc.dram_tensor(
        "ag_v1_out", [N_CORES * 128, GROUP, H // 2, HD + 1], BF16,
        addr_space="Shared",
    )
    ag_v2_in = nc.dram_tensor("ag_v2_in", [128, GROUP, H // 2, HD + 1], BF16)
    ag_v2_out = nc.dram_tensor(
        "ag_v2_out", [N_CORES * 128, GROUP, H // 2, HD + 1], BF16,
        addr_space="Shared",
    )
    groups = [[0, 1, 2, 3, 4, 5, 6, 7]]

    with tile.TileContext(nc) as tc:
        import contextlib

        with contextlib.ExitStack() as stk:
            # ---- persistent pools --------------------------------------
            w_pool = stk.enter_context(tc.tile_pool(name="w", bufs=1))
            kt_pool = stk.enter_context(tc.tile_pool(name="kt", bufs=1))
            vp_pool = stk.enter_context(tc.tile_pool(name="vp", bufs=1))
            q_pool = stk.enter_context(tc.tile_pool(name="q", bufs=1))
            ao_pool = stk.enter_context(tc.tile_pool(name="ao", bufs=1))
            const_pool = stk.enter_context(tc.tile_pool(name="const", bufs=1))

            # 1024-wide weight tiles, 16 rotating slots: wk(8)+wv(8) live
            # first; wq rotates into wk's slots (dead after the K wave) and
            # wp into wv's (dead after the V wave)
            wk_t = [w_pool.tile([128, D], BF16, name=f"wk{k}", tag="w1024", bufs=16)
                    for k in range(N_KT)]
            wv_t = [w_pool.tile([128, D], BF16, name=f"wv{k}", tag="w1024", bufs=16)
                    for k in range(N_KT)]
            wq_t = [w_pool.tile([128, D], BF16, name=f"wq{k}", tag="w1024", bufs=16)
                    for k in range(N_KT)]

            # kT kept fp8 end-to-end: the QK matmul takes fp8 stationary with
            # bf16 moving directly, and fp8 halves both SBUF and AG_K bytes.
            # One tile [128, pair, key] so the payload store and each gather
            # load is a single DMA.
            ktall = kt_pool.tile([128, PAIRS, S], FP8, name="ktall", tag="ktall")
            # V likewise one tile per rank-block: [128, m, head, hd+ones]
            vpb = [vp_pool.tile([128, GROUP, H, HD + 1], BF16, name=f"vpb{b}",
                                tag=f"vpb{b}")
                   for b in range(GROUP)]
            qA = [q_pool.tile([128, S_OWN], BF16, name=f"qA{p}", tag=f"qA{p}")
                  for p in range(PAIRS)]
            qB = [q_pool.tile([128, S_OWN], BF16, name=f"qB{p}", tag=f"qB{p}")
                  for p in range(PAIRS)]
            aoT = [ao_pool.tile([128, S_OWN], BF16, name=f"ao{p}", tag=f"ao{p}")
                   for p in range(PAIRS)]

            ones_t = const_pool.tile([128, 128], FP32R, tag="ones")
            ones16_t = const_pool.tile([128, H], BF16, tag="ones16")
            fc_t = const_pool.tile([128, FC_W], FP32, tag="fc")
            bv_t = const_pool.tile([128, D], FP32, tag="bv")
            bp_t = const_pool.tile([128, D], FP32, tag="bp")
            actw_t = const_pool.tile([128, 1], FP32, tag="actw")

            # constants via memset -- no DMA wait, so the PE warm-up starts
            # immediately at kernel entry instead of after the DMA ramp
            # (memset only accepts plain dtypes; fp32r/bf16 come via copies)
            ones32_t = const_pool.tile([128, 128], FP32, tag="ones32")
            nc.vector.memset(ones32_t[:], 1.0)
            nc.vector.tensor_copy(ones_t[:], ones32_t[:])
            nc.vector.tensor_copy(ones16_t[:], ones32_t[:, 0:H])
            nc.gpsimd.dma_start(fc_t[:], fc_d.ap())

            # ---- PE warm-up + ACT exp-table preload during DMA ramp ----
            with tc.tile_pool(name="warm", bufs=1, space="PSUM") as warm_pool:
                wps = warm_pool.tile([128, 128], FP32, tag="warm")
                for w in range(16):
                    nc.tensor.matmul(
                        wps[:], ones_t[:], ones_t[:],
                        start=True, stop=True, skip_group_check=True,
                    )
            with (
                tc.tile_pool(name="xp", bufs=1) as x_pool,
                tc.tile_pool(name="psA", bufs=1, space="PSUM") as psA,
            ):
                xq_t = [x_pool.tile([128, S_OWN], BF16, name=f"xq{k}", tag=f"xq{k}")
                        for k in range(N_KT)]
                # ones columns of the own V tiles (ride into the payload)
                for m in range(4):
                    nc.vector.tensor_copy(
                        vpb[0][:, m, :, HD : HD + 1], ones16_t[:]
                    )

                # input streams: xq + wv on sync, wk + wq (+bv/bp, wp) on scalar
                for k in range(N_KT):
                    sl = slice(k * 128, (k + 1) * 128)
                    nc.sync.dma_start(xq_t[k][:], xq_d.ap()[sl, :])
                    nc.scalar.dma_start(wk_t[k][:], wk_d.ap()[sl, :])
                for k in range(N_KT):
                    sl = slice(k * 128, (k + 1) * 128)
                    nc.scalar.dma_start(wv_t[k][:], wv_d.ap()[sl, :])
                for k in range(N_KT):
                    sl = slice(k * 128, (k + 1) * 128)
                    nc.scalar.dma_start(wq_t[k][:], wq_d.ap()[sl, :])
                nc.scalar.dma_start(bv_t[:], bv_d.ap())
                nc.scalar.dma_start(bp_t[:], bp_d.ap())
                nc.scalar.activation(
                    actw_t[:], fc_t[:, FC_MA : FC_MA + 1],
                    mybir.ActivationFunctionType.Exp, scale=0.125,
                )

                ps = [psA.tile([128, 512], FP32, name=f"psA{g}", tag=f"psA{g}")
                      for g in range(8)]

                # -- K wave: kT_part[j] = wk.T @ xq; evac into kT[j][:, 0:512]
                for k in range(N_KT):
                    for j in range(8):
                        nc.tensor.matmul(
                            ps[j][:],
                            wk_t[k][:, j * 128 : (j + 1) * 128],
                            xq_t[k][:],
                            start=(k == 0), stop=(k == N_KT - 1),
                        )
                for j in range(8):
                    nc.vector.tensor_scalar(
                        ktall[:, j, 0:S_OWN], ps[j][:],
                        fc_t[:, FC_BK + j : FC_BK + j + 1], None,
                        mybir.AluOpType.add,
                    )
                nc.sync.dma_start(ag_k_in.ap(), ktall[:, :, 0:S_OWN])
                nc.gpsimd.collective_compute(
                    "AllGather", mybir.AluOpType.bypass, replica_groups=groups,
                    ins=[ag_k_in.ap()], outs=[ag_k_out.ap()],
                )

                # -- Q wave: qT_own[j] = wq.T @ xq, masked/padded halves --
                # (before the V wave: this unblocks sweep-1's exp chain on
                # ACT ~10us earlier; AG_V's input is still ready before the
                # CC engine finishes AG_K)
                # j-outer so ps_q[0] completes after 8 matmuls (its K-evac
                # dependency clears first) instead of at wave end -- the PE
                # never waits for the whole K evacuation drain
                ps_q = [psA.tile([128, 512], FP32, name=f"psQ{g}", tag=f"psA{g}")
                        for g in range(8)]
                for j in range(8):
                    for k in range(N_KT):
                        nc.tensor.matmul(
                            ps_q[j][:],
                            wq_t[k][:, j * 128 : (j + 1) * 128],
                            xq_t[k][:],
                            start=(k == 0), stop=(k == N_KT - 1),
                        )
                for j in range(8):
                    nc.vector.tensor_scalar(
                        qA[j][:], ps_q[j][:],
                        fc_t[:, FC_MA : FC_MA + 1],
                        fc_t[:, FC_BQA + j : FC_BQA + j + 1],
                        mybir.AluOpType.mult, mybir.AluOpType.add,
                    )
                    nc.vector.tensor_scalar(
                        qB[j][:], ps_q[j][:],
                        fc_t[:, FC_MB : FC_MB + 1],
                        fc_t[:, FC_BQB + j : FC_BQB + j + 1],
                        mybir.AluOpType.mult, mybir.AluOpType.add,
                    )

                # -- V wave: V_own[m] = xq.T @ wv -------------------------
                # (m,nb)-outer for the same reason: early banks evac while
                # later banks still accumulate, so the phase-B PSUM pools
                # (which reuse these banks) open right at wave end
                ps_v = [psA.tile([128, 512], FP32, name=f"psV{g}", tag=f"psA{g}")
                        for g in range(8)]
                for m in range(4):
                    for nb in range(2):
                        for k in range(N_KT):
                            nc.tensor.matmul(
                                ps_v[m * 2 + nb][:],
                                xq_t[k][:, m * 128 : (m + 1) * 128],
                                wv_t[k][:, nb * 512 : (nb + 1) * 512],
                                start=(k == 0), stop=(k == N_KT - 1),
                            )
                for m in range(4):
                    for nb in range(2):
                        nc.vector.tensor_tensor(
                            vpb[0][:, m, nb * 8 : (nb + 1) * 8, 0:HD],
                            ps_v[m * 2 + nb][:],
                            bv_t[:, nb * 512 : (nb + 1) * 512],
                            mybir.AluOpType.add,
                        )
                # V stores on scalar: on sync the scheduler parks them behind
                # the kT gather loads (which wait for AG_K), delaying AG_V's
                # input-readiness by ~40us
                nc.scalar.dma_start(ag_v1_in.ap(), vpb[0][:, :, 0 : H // 2, :])
                nc.scalar.dma_start(ag_v2_in.ap(), vpb[0][:, :, H // 2 : H, :])
                nc.gpsimd.collective_compute(
                    "AllGather", mybir.AluOpType.bypass, replica_groups=groups,
                    ins=[ag_v1_in.ap()], outs=[ag_v1_out.ap()],
                )
                nc.gpsimd.collective_compute(
                    "AllGather", mybir.AluOpType.bypass, replica_groups=groups,
                    ins=[ag_v2_in.ap()], outs=[ag_v2_out.ap()],
                )

                # -- gathered loads (runtime-rotated so tiles 0..3 = own) --
                # gathered loads from the 8-rank outputs: this core's group
                # peers are global ranks base + ((pid%4)+b)%4. One DMA per
                # rank-block. kT loads on sync, vp loads on gpsimd: if they
                # share a queue they share a DMA-completion semaphore, and
                # the Tensor engine's wait for the kT writes then rounds up
                # to "all loads done", gating sweep-2's QK on AG_V.
                pid_s = nc.sync.partition_id()
                rank_s = pid_s % GROUP
                base_s = (pid_s // GROUP) * GROUP
                for b in range(1, GROUP):
                    rg = (rank_s + b) % GROUP
                    row0 = (base_s + rg) * 128
                    nc.sync.dma_start(
                        ktall[:, 0:2, b * S_OWN : (b + 1) * S_OWN],
                        ag_k_out.ap()[bass.ds(row0, 128), 0:2, :],
                    )
                for b in range(1, GROUP):
                    rg = (rank_s + b) % GROUP
                    row0 = (base_s + rg) * 128
                    nc.sync.dma_start(
                        ktall[:, 2:PAIRS, b * S_OWN : (b + 1) * S_OWN],
                        ag_k_out.ap()[bass.ds(row0, 128), 2:PAIRS, :],
                    )
                pid_g = nc.gpsimd.partition_id()
                rank_g = pid_g % GROUP
                base_g = (pid_g // GROUP) * GROUP
                for b in range(1, GROUP):
                    rg = (rank_g + b) % GROUP
                    row0 = (base_g + rg) * 128
                    nc.gpsimd.dma_start(
                        vpb[b][:, :, 0:4, :],
                        ag_v1_out.ap()[bass.ds(row0, 128), :, 0:4, :],
                    )
                for b in range(1, GROUP):
                    rg = (rank_g + b) % GROUP
                    row0 = (base_g + rg) * 128
                    nc.gpsimd.dma_start(
                        vpb[b][:, :, 4 : H // 2, :],
                        ag_v1_out.ap()[bass.ds(row0, 128), :, 4 : H // 2, :],
                    )
                for b in range(1, GROUP):
                    rg = (rank_g + b) % GROUP
                    row0 = (base_g + rg) * 128
                    nc.gpsimd.dma_start(
                        vpb[b][:, :, H // 2 : H, :],
                        ag_v2_out.ap()[bass.ds(row0, 128), :, :, :],
                    )

            # wp prefetch (rotates into w1024 slots)
            wp_t = [w_pool.tile([128, D], BF16, name=f"wp{k}", tag="w1024", bufs=16)
                    for k in range(N_KT)]
            for k in range(N_KT):
                nc.scalar.dma_start(wp_t[k][:], wp_d.ap()[k * 128 : (k + 1) * 128, :])

            # ---- phase B: attention ------------------------------------
            # sweep 1: own key tiles (0..3) for every pair -- runs while the
            # gathers are in flight; PSUM accs partial-evac'd to SBUF.
            # sweep 2: gathered tiles (4..15), partial re-added, normalize.
            with (
                tc.tile_pool(name="p", bufs=30) as p_pool,
                tc.tile_pool(name="part", bufs=1) as part_pool,
                tc.tile_pool(name="rr", bufs=1) as rr_pool,
                tc.tile_pool(name="rcp", bufs=1) as rcp_pool,
                tc.tile_pool(name="asb", bufs=4) as asb_pool,
                tc.tile_pool(name="psc", bufs=2, space="PSUM") as ps_sc,
                tc.tile_pool(name="pacc", bufs=2, space="PSUM") as ps_acc,
            ):
                part_a = [part_pool.tile([128, S_OWN], BF16, name=f"pa{p}", tag=f"pa{p}")
                          for p in range(PAIRS)]
                part_b = [part_pool.tile([128, S_OWN], BF16, name=f"pb{p}", tag=f"pb{p}")
                          for p in range(PAIRS)]

                def qk_block(p, tp):
                    t0, t1 = 2 * tp, 2 * tp + 1
                    sca = ps_sc.tile([128, 1024], FP32, tag="sc", name=f"sca{p}_{tp}")
                    scb = ps_sc.tile([128, 1024], FP32, tag="sc", name=f"scb{p}_{tp}")
                    for ti, t in enumerate((t0, t1)):
                        tsl = slice(t * 128, (t + 1) * 128)
                        usl = slice(ti * 512, (ti + 1) * 512)
                        nc.tensor.matmul(
                            sca[:, usl], ktall[:, p, tsl], qA[p][:],
                            start=True, stop=True,
                        )
                    for ti, t in enumerate((t0, t1)):
                        tsl = slice(t * 128, (t + 1) * 128)
                        usl = slice(ti * 512, (ti + 1) * 512)
                        nc.tensor.matmul(
                            scb[:, usl], ktall[:, p, tsl], qB[p][:],
                            start=True, stop=True,
                        )
                    if p == 3 and tp < 6:
                        # pair 3's early probs tiles come from the 8 wq
                        # weight slots (dead after the Q wave, same shape):
                        # extends the V-independent QK/exp lookahead to ~3.5
                        # pairs without new SBUF
                        pa = w_pool.tile([128, D], BF16, name=f"pwa{tp}",
                                         tag="w1024", bufs=16)
                        pb = w_pool.tile([128, D], BF16, name=f"pwb{tp}",
                                         tag="w1024", bufs=16)
                    else:
                        pa = p_pool.tile([128, 1024], BF16, tag="pt",
                                         name=f"pta{p}_{tp}")
                        pb = p_pool.tile([128, 1024], BF16, tag="pt",
                                         name=f"ptb{p}_{tp}")
                    nc.scalar.activation(
                        pa[:], sca[:], mybir.ActivationFunctionType.Exp, scale=0.125
                    )
                    nc.scalar.activation(
                        pb[:], scb[:], mybir.ActivationFunctionType.Exp, scale=0.125
                    )
                    return pa, pb

                def pv_block(p, tp, pa, pb, acc_a, acc_b, first_tp, last_tp):
                    t0, t1 = 2 * tp, 2 * tp + 1
                    for ti, t in enumerate((t0, t1)):
                        usl = slice(ti * 512, (ti + 1) * 512)
                        first = tp == first_tp and ti == 0
                        last = tp == last_tp and ti == 1
                        nc.tensor.matmul(
                            acc_a[0:65, :],
                            vpb[t // 4][:, t % 4, 2 * p : 2 * p + 1, :],
                            pa[:, usl], start=first, stop=last,
                        )
                        nc.tensor.matmul(
                            acc_b[0:65, :],
                            vpb[t // 4][:, t % 4, 2 * p + 1 : 2 * p + 2, :],
                            pb[:, usl], start=first, stop=last,
                        )

                def attn_block(p, tp, acc_a, acc_b, first_tp, last_tp):
                    pa, pb = qk_block(p, tp)
                    pv_block(p, tp, pa, pb, acc_a, acc_b, first_tp, last_tp)

                # sweep 1: local key tiles
                for p in range(PAIRS):
                    acc_a = ps_acc.tile([128, S_OWN], FP32, tag="acca", name=f"a1a{p}")
                    acc_b = ps_acc.tile([128, S_OWN], FP32, tag="accb", name=f"a1b{p}")
                    for tp in range(2):
                        attn_block(p, tp, acc_a, acc_b, 0, 1)
                    nc.vector.tensor_copy(part_a[p][0:65, :], acc_a[0:65, :])
                    nc.vector.tensor_copy(part_b[p][0:65, :], acc_b[0:65, :])

                # sweep 2: gathered key tiles + combine + normalize.
                # Each pair's normalize is emitted after the NEXT pair's first
                # block so the PE/ACT pipeline is fed across pair boundaries.
                def emit_normalize(p, acc_a, acc_b):
                    # combine with sweep-1 partials into SBUF, freeing the
                    # PSUM accs immediately for the next pair's PV
                    asb_a = asb_pool.tile([128, S_OWN], FP32, tag="asb",
                                          name=f"asba{p}")
                    asb_b = asb_pool.tile([128, S_OWN], FP32, tag="asb",
                                          name=f"asbb{p}")
                    nc.vector.tensor_tensor(
                        asb_a[0:65, :], acc_a[0:65, :], part_a[p][0:65, :],
                        mybir.AluOpType.add,
                    )
                    nc.vector.tensor_tensor(
                        asb_b[0:65, :], acc_b[0:65, :], part_b[p][0:65, :],
                        mybir.AluOpType.add,
                    )
                    # normalize both halves with one reciprocal
                    rrow = rr_pool.tile([1, 1024], FP32R, tag="rrow")
                    nc.vector.tensor_copy(rrow[:, 0:512], asb_a[64:65, :])
                    nc.vector.tensor_copy(rrow[:, 512:1024], asb_b[64:65, :])
                    rb = ps_sc.tile([64, 1024], FP32, tag="sc", name=f"rb{p}")
                    nc.tensor.matmul(
                        rb[:, 0:512], ones_t[0:1, 0:64], rrow[:, 0:512],
                        start=True, stop=True,
                    )
                    nc.tensor.matmul(
                        rb[:, 512:1024], ones_t[0:1, 0:64], rrow[:, 512:1024],
                        start=True, stop=True,
                    )
                    rc = rcp_pool.tile([64, 1024], FP32, tag="rc")
                    nc.vector.reciprocal_approx_fast(rc[:], rb[:])
                    nc.vector.tensor_tensor(
                        aoT[p][0:64, :], asb_a[0:64, :], rc[:, 0:512],
                        mybir.AluOpType.mult,
                    )
                    nc.vector.tensor_tensor(
                        aoT[p][64:128, :], asb_b[0:64, :], rc[:, 512:1024],
                        mybir.AluOpType.mult,
                    )

                # Software-pipelined: emit each pair's full QK+exp chain one
                # pair ahead of its PV chain, so the in-order PE queue never
                # parks on a PV that's waiting for the gathered V tiles --
                # the next pair's QK matmuls (and their exps on ACT) proceed.
                probs = {}

                def qk_chain(p):
                    probs[p] = [
                        (tp,) + qk_block(p, tp) for tp in range(2, N_SK // 2)
                    ]

                def pv_chain(p):
                    acc_a = ps_acc.tile([128, S_OWN], FP32, tag="acca", name=f"a2a{p}")
                    acc_b = ps_acc.tile([128, S_OWN], FP32, tag="accb", name=f"a2b{p}")
                    for tp, pa, pb in probs.pop(p):
                        pv_block(p, tp, pa, pb, acc_a, acc_b, 2, N_SK // 2 - 1)
                    return (p, acc_a, acc_b)

                # The PV chains carry explicit schedule-time floors: without
                # them the Tile scheduler interleaves pair 0's PV matmuls
                # right behind its QK matmuls in the in-order PE queue, and
                # the whole engine parks on the V gather. The floors push
                # every PV chain behind the QK/exp chains that can run
                # V-independently.
                # One flat floor just past the expected V arrival: the
                # p-slot WAR dependencies then self-pace the interleave
                # (qk3 can only schedule after pv0 frees its slots, etc.),
                # and the tail PV/normalize chains are not pushed late by
                # graduated floors.
                qk_chain(0)
                qk_chain(1)
                qk_chain(2)
                with tc.tile_wait_until(0.136):
                    pending = pv_chain(0)
                for p in range(3, PAIRS):
                    qk_chain(p)
                    with tc.tile_wait_until(0.136):
                        emit_normalize(*pending)
                        pending = pv_chain(p - 2)
                with tc.tile_wait_until(0.136):
                    emit_normalize(*pending)
                    pending = pv_chain(PAIRS - 2)
                    emit_normalize(*pending)
                    # pre-start two projection chunks' kd<=6 accumulation in
                    # the spare acc-tag PSUM slots (free after norm6): the
                    # matmuls fill the ACT-paced bubbles before pv7, and
                    # only their kd=7 step remains for the tail. sc slots
                    # stay free for norm7's broadcast.
                    pre = []
                    for i in range(2):
                        pd = ps_acc.tile([128, S_OWN], FP32,
                                         tag=("acca" if i == 0 else "accb"),
                                         name=f"psDpre{i}")
                        for kd in range(N_KT - 1):
                            nc.tensor.matmul(
                                pd[:],
                                aoT[kd][:, 0:128],
                                wp_t[kd][:, i * 512 : (i + 1) * 512],
                                start=(kd == 0), stop=False,
                            )
                        pre.append(pd)
                    pending = pv_chain(PAIRS - 1)
                    emit_normalize(*pending)

                # ---- phase C: local output projection ------------------
                # Emitted inside phase B, reusing sc-tag PSUM slots and
                # asb-tag SBUF slots: no pool-boundary barrier.
                for i, pd in enumerate(pre):
                    nsl = slice(i * 512, (i + 1) * 512)
                    nc.tensor.matmul(
                        pd[:], aoT[N_KT - 1][:, 0:128],
                        wp_t[N_KT - 1][:, nsl],
                        start=False, stop=True,
                    )
                    ot = asb_pool.tile([128, 512], BF16, tag="asb",
                                       name=f"ot0_{i}")
                    nc.vector.tensor_tensor(
                        ot[:], pd[:], bp_t[:, nsl], mybir.AluOpType.add
                    )
                    nc.sync.dma_start(out_d.ap()[0:128, nsl], ot[:])
                # m=1 chunks ride the acc slots freed by pv7/norm7: the
                # projection pipeline then rotates 4 PSUM slots instead of
                # 2, removing the chunk-boundary DVE-wait stalls
                for nb in range(2):
                    nsl = slice(nb * 512, (nb + 1) * 512)
                    pd = ps_acc.tile([128, S_OWN], FP32,
                                     tag=("acca" if nb == 0 else "accb"),
                                     name=f"psDm1_{nb}")
                    for kd in range(N_KT):
                        nc.tensor.matmul(
                            pd[:],
                            aoT[kd][:, 128:256],
                            wp_t[kd][:, nsl],
                            start=(kd == 0), stop=(kd == N_KT - 1),
                        )
                    ot = asb_pool.tile([128, 512], BF16, tag="asb",
                                       name=f"otm1_{nb}")
                    nc.vector.tensor_tensor(
                        ot[:], pd[:], bp_t[:, nsl], mybir.AluOpType.add
                    )
                    nc.sync.dma_start(out_d.ap()[128:256, nsl], ot[:])
                for m in range(2, 4):
                    msl = slice(m * 128, (m + 1) * 128)
                    for nb in range(2):
                        nsl = slice(nb * 512, (nb + 1) * 512)
                        pd = ps_sc.tile([128, 512], FP32, tag="sc",
                                        name=f"psD{m * 2 + nb}")
                        for kd in range(N_KT):
                            nc.tensor.matmul(
                                pd[:],
                                aoT[kd][:, msl],
                                wp_t[kd][:, nsl],
                                start=(kd == 0), stop=(kd == N_KT - 1),
                            )
                        ot = asb_pool.tile([128, 512], BF16, tag="asb",
                                           name=f"ot{m}_{nb}")
                        nc.vector.tensor_tensor(
                            ot[:], pd[:], bp_t[:, nsl], mybir.AluOpType.add
                        )
                        nc.sync.dma_start(out_d.ap()[msl, nsl], ot[:])

    nc.compile()
    return nc


def _get_program():
    global _compiled
    if _compiled is None:
        _compiled = _build()
    return _compiled


def _make_in_maps(x, w_qkv, b_qkv, w_proj, b_proj):
    x = np.asarray(x, dtype=np.float32)
    w_qkv = np.asarray(w_qkv, dtype=np.float32)
    b_qkv = np.asarray(b_qkv, dtype=np.float32)
    w_proj = np.asarray(w_proj, dtype=np.float32)
    b_proj = np.asarray(b_proj, dtype=np.float32)

    wq16 = np.ascontiguousarray(w_qkv[:, 0:D]).astype(BF16_NP)
    wk16 = np.ascontiguousarray(w_qkv[:, D : 2 * D]).astype(BF16_NP)
    wv16 = np.ascontiguousarray(w_qkv[:, 2 * D : 3 * D]).astype(BF16_NP)
    wp16 = w_proj.astype(BF16_NP)

    fc = np.zeros((128, FC_W), dtype=np.float32)
    bq = b_qkv[0:D]
    bk = b_qkv[D : 2 * D]
    for j in range(PAIRS):
        fc[0:64, FC_BQA + j] = bq[j * 128 : j * 128 + 64]
        fc[64:128, FC_BQB + j] = bq[j * 128 + 64 : (j + 1) * 128]
        fc[:, FC_BK + j] = bk[j * 128 : (j + 1) * 128]
    fc[0:64, FC_MA] = 1.0
    fc[64:128, FC_MB] = 1.0
    bv_b = np.ascontiguousarray(
        np.broadcast_to(b_qkv[2 * D : 3 * D].reshape(1, D), (128, D))
    )
    bp_b = np.ascontiguousarray(np.broadcast_to(b_proj.reshape(1, D), (128, D)))

    xT = [np.ascontiguousarray(x[g].T).astype(BF16_NP) for g in range(B)]
    in_maps = []
    for c in range(N_CORES):
        g, r = c // GROUP, c % GROUP
        in_maps.append(
            {
                "xq": np.ascontiguousarray(
                    xT[g][:, r * S_OWN : (r + 1) * S_OWN]
                ),
                "wq": wq16,
                "wk": wk16,
                "wv": wv16,
                "wp": wp16,
                "fc": fc,
                "bv": bv_b,
                "bp": bp_b,
            }
        )
    return in_maps


def _assemble(results):
    out = np.empty((B, S, D), dtype=np.float32)
    for c in range(N_CORES):
        g, r = c // GROUP, c % GROUP
        out[g, r * S_OWN : (r + 1) * S_OWN, :] = results[c]["out"]
    return out


def kernel(x, w_qkv, b_qkv, w_proj, b_proj):
    nc = _get_program()
    in_maps = _make_in_maps(x, w_qkv, b_qkv, w_proj, b_proj)
    res = run_bass_kernel_spmd(nc, in_maps, list(range(N_CORES)))
    return _assemble(res.results)

